# revision 15
# baseline (speedup 1.0000x reference)
"""Trainium2 Bass kernel for causal multi-head attention with full-dim rotary.

Computes, for inputs q,k,v [B=2, L=2048, D=1024] and weights Wq/Wk/Wv/Wo [D,D]:
    Q = rope(q @ Wq.T + bq); K = rope(k @ Wk.T + bk); V = v @ Wv.T + bv
    out = softmax_causal(Qh Kh^T / sqrt(dh)) Vh  (H=16 heads, dh=64)
    y = out @ Wo.T + bo

Sharding: 8 cores = (batch b in {0,1}) x (block of 4 heads). Each core computes
its 4 heads' Q/K/V projections (256 output features), runs causal flash
attention in S^T orientation, and emits a partial output projection y [L, D].
The host sums the 4 partials per batch and adds the bias correction row.

v2 architecture (software-pipelined super-steps over 512-row chunks):
  pass m attention (ACT-bound exp stream) overlaps the PE-side projection of
  chunk m+1 and out-projection of rows from pass m-1, woven into the round
  loop so the tensor engine never idles long enough to re-throttle (HAM).
  Both heads of a pair share one exp instruction ([128, 2, w] across two PSUM
  banks). All matmuls are bf16 (fp8 fails the max-error budget here). The
  softmax denominator is fused as a ones-column of V'; the causal mask is a
  -1000 pre-exp bias added onto the diagonal score blocks; exp carries a -3
  shift (cancels in the softmax ratio).
"""

import sys
import functools
import numpy as np

try:
    import concourse.bass as bass
except ImportError:  # fresh grading dir: concourse lives in the container image
    sys.path.insert(0, "/opt/trn_rl_repo")
    import concourse.bass as bass

import ml_dtypes
import concourse.mybir as mybir
import concourse.tile as tile
from concourse import bacc
from concourse.bass_utils import run_bass_kernel_spmd

BF16 = mybir.dt.bfloat16
F32 = mybir.dt.float32
FP8 = mybir.dt.float8e4
AF = mybir.ActivationFunctionType
DR = mybir.MatmulPerfMode.DoubleRow

B, D, H, DH = 2, 1024, 16, 64
P = 128
KT = D // P            # 8 contraction tiles for the projections
HPC = 4                # heads per core
NPR = 2                # head-pairs per core
N_CORES = 8
MAXPOS = 10000.0
MC = 512               # chunk / pass width
TPC = MC // P          # key tiles per chunk
VW = 72                # padded per-head V' width (64 dims + ones + pad)
EXP_SHIFT = -3.0
MASK_BIAS = -1000.0


def build_program(L, qk_bias=False, debug=False):
    NCH = L // MC
    NT = L // P

    nc = bacc.Bacc(None)
    xq = nc.declare_dram_parameter("xq", [NCH, D, MC], BF16, isOutput=False)
    xk = nc.declare_dram_parameter("xk", [NCH, D, MC], BF16, isOutput=False)
    xv = nc.declare_dram_parameter("xv", [NCH, D, MC], BF16, isOutput=False)
    wq = nc.declare_dram_parameter("wq", [D, 2 * P], BF16, isOutput=False)
    wk = nc.declare_dram_parameter("wk", [D, 2 * P], BF16, isOutput=False)
    wv = nc.declare_dram_parameter("wv", [D, 2 * P], BF16, isOutput=False)
    wo = nc.declare_dram_parameter("wo", [2 * P, D], BF16, isOutput=False)
    ctab = nc.declare_dram_parameter("ctab", [NCH, NPR, P, MC], BF16, isOutput=False)
    stab = nc.declare_dram_parameter("stab", [NCH, NPR, P, MC], BF16, isOutput=False)
    if qk_bias:
        rqt = nc.declare_dram_parameter("rqt", [NCH, NPR, P, MC], BF16, isOutput=False)
        rkt = nc.declare_dram_parameter("rkt", [NCH, NPR, P, MC], BF16, isOutput=False)
    trib = nc.declare_dram_parameter("trib", [P, 2, P], BF16, isOutput=False)
    y = nc.declare_dram_parameter("y", [L, D], BF16, isOutput=True)
    if debug:
        dot = nc.declare_dram_parameter("dot", [NCH, P, NPR, MC], BF16,
                                        isOutput=True)
        dr32 = nc.declare_dram_parameter("dr32", [NCH, NPR, 2, MC], F32,
                                         isOutput=True)

    with tile.TileContext(nc) as tc:
        from contextlib import ExitStack

        with ExitStack() as ctx:
            consts = ctx.enter_context(tc.tile_pool(name="consts", bufs=1))
            qk_sb = ctx.enter_context(tc.tile_pool(name="qk_sb", bufs=1))
            xin = ctx.enter_context(tc.tile_pool(name="xin", bufs=2))
            tmp = ctx.enter_context(tc.tile_pool(name="tmp", bufs=3))
            pts = ctx.enter_context(tc.tile_pool(name="pts", bufs=3))
            small = ctx.enter_context(tc.tile_pool(name="small", bufs=2))
            pss = ctx.enter_context(tc.tile_pool(name="pss", bufs=1, space="PSUM"))
            pop = ctx.enter_context(tc.tile_pool(name="pop", bufs=1, space="PSUM"))

            # ---- constant/persistent tiles ----
            wq_sb = consts.tile([P, KT, 2 * P], BF16, tag="wq")
            wk_sb = consts.tile([P, KT, 2 * P], BF16, tag="wk")
            wv_sb = consts.tile([P, KT, 2 * P], BF16, tag="wv")
            wo_sb = consts.tile([P, NPR, D], BF16, tag="wo")
            trib_sb = consts.tile([P, 2, P], BF16, tag="trib")
            bias_sb = consts.tile([P, 1], F32, tag="bias")
            nc.vector.memset(bias_sb[:], EXP_SHIFT)

            c_sb = [consts.tile([P, NPR, MC], BF16, tag=f"c{m}", name=f"c{m}")
                    for m in range(NCH)]
            s_sb = [consts.tile([P, NPR, MC], BF16, tag=f"s{m}", name=f"s{m}")
                    for m in range(NCH)]
            if qk_bias:
                rq_sb = [consts.tile([P, NPR, MC], BF16, tag=f"rq{m}", name=f"rq{m}")
                         for m in range(NCH)]
                rk_sb = [consts.tile([P, NPR, MC], BF16, tag=f"rk{m}", name=f"rk{m}")
                         for m in range(NCH)]

            QT = [[qk_sb.tile([P, MC], BF16, tag=f"QT{pr}_{m}", name=f"QT{pr}_{m}")
                   for m in range(NCH)] for pr in range(NPR)]
            KTt = [[qk_sb.tile([P, MC], BF16, tag=f"KT{pr}_{m}", name=f"KT{pr}_{m}")
                    for m in range(NCH)] for pr in range(NPR)]
            OT = [qk_sb.tile([P, NPR, MC], BF16, tag=f"OT_{m}", name=f"OT_{m}")
                  for m in range(NCH)]
            # V' per chunk: [keys, tile, head, 64 dims + ones]
            Vp = [qk_sb.tile([P, TPC, HPC, DH + 1], BF16, tag=f"Vp{m}",
                             name=f"Vp{m}") for m in range(NCH)]
            for m in range(NCH):
                nc.vector.memset(Vp[m][:, :, :, DH : DH + 1], 1.0)

            # a<->b half swap within each 32-partition quadrant
            SWAP = [(i + 16) % 32 for i in range(32)]

            # ---------------- emission helpers ----------------
            def dma_chunk(m):
                xq_t = xin.tile([P, KT, MC], BF16, tag="xq", name=f"xq{m}")
                xk_t = xin.tile([P, KT, MC], BF16, tag="xk", name=f"xk{m}")
                xv_t = xin.tile([P, KT, MC], BF16, tag="xv", name=f"xv{m}")
                nc.sync.dma_start(xq_t[:], xq[m].rearrange("(kt p) c -> p kt c", p=P))
                nc.sync.dma_start(c_sb[m][:], ctab[m].rearrange("pr p c -> p pr c"))
                nc.sync.dma_start(s_sb[m][:], stab[m].rearrange("pr p c -> p pr c"))
                nc.sync.dma_start(xk_t[:], xk[m].rearrange("(kt p) c -> p kt c", p=P))
                nc.sync.dma_start(xv_t[:], xv[m].rearrange("(kt p) c -> p kt c", p=P))
                if qk_bias:
                    nc.sync.dma_start(rq_sb[m][:],
                                      rqt[m].rearrange("pr p c -> p pr c"))
                    nc.sync.dma_start(rk_sb[m][:],
                                      rkt[m].rearrange("pr p c -> p pr c"))
                return xq_t, xk_t, xv_t

            rot = [0]  # pss-slot borrow rotation (proj + outproj psy)

            def proj_units(m, xq_t, xk_t, xv_t):
                """PE work units projecting chunk m into QT/KTt/Vp.
                rope: sb = bf16(ps); t2s = shuffle(sb*sin); QT = sb*cos + t2s
                (multiplies and add on gpsimd, shuffle on DVE)."""
                units = []
                for name, w_sb, x_t, dst in (
                    ("q", wq_sb, xq_t, QT),
                    ("k", wk_sb, xk_t, KTt),
                ):
                    for pr in range(NPR):
                        def u(pr=pr, name=name, w_sb=w_sb, x_t=x_t, dst=dst):
                            ps = pss.tile([P, MC], F32, tag=f"s{rot[0] % 2}",
                                          name=f"pj{name}{m}_{pr}")
                            rot[0] += 1
                            for kt in range(KT):
                                nc.tensor.matmul(
                                    ps[:],
                                    lhsT=w_sb[:, kt, pr * P : pr * P + P],
                                    rhs=x_t[:, kt, :],
                                    start=(kt == 0),
                                    stop=(kt == KT - 1),
                                    skip_group_check=True,
                                )
                            sb = tmp.tile([P, MC], BF16, tag="t0")
                            t2 = tmp.tile([P, MC], BF16, tag="t2")
                            t2s = tmp.tile([P, MC], BF16, tag="t2s")
                            nc.vector.tensor_copy(sb[:], ps[:])
                            nc.gpsimd.tensor_mul(t2[:], sb[:], s_sb[m][:, pr, :])
                            nc.vector.stream_shuffle(t2s[:], t2[:], SWAP)
                            t1 = tmp.tile([P, MC], BF16, tag="t1")
                            nc.gpsimd.tensor_mul(t1[:], sb[:], c_sb[m][:, pr, :])
                            if qk_bias:
                                r_sb = rq_sb if name == "q" else rk_sb
                                t3 = tmp.tile([P, MC], BF16, tag="t3")
                                nc.gpsimd.tensor_add(t3[:], t1[:], t2s[:])
                                nc.gpsimd.tensor_add(
                                    dst[pr][m][:], t3[:], r_sb[m][:, pr, :]
                                )
                            else:
                                nc.gpsimd.tensor_add(dst[pr][m][:], t1[:], t2s[:])
                        units.append(u)
                for msub in range(TPC):
                    def uv(msub=msub):
                        ps = pss.tile([P, MC], F32, tag=f"s{rot[0] % 2}",
                                      name=f"pjv{m}_{msub}")
                        rot[0] += 1
                        psv = ps[:, 0 : 2 * P]
                        for kt in range(KT):
                            nc.tensor.matmul(
                                psv,
                                lhsT=xv_t[:, kt, msub * P : msub * P + P],
                                rhs=wv_sb[:, kt, :],
                                start=(kt == 0),
                                stop=(kt == KT - 1),
                                skip_group_check=True,
                            )
                        nc.vector.tensor_copy(
                            Vp[m][:, msub, :, 0:DH],
                            psv.rearrange("p (h d) -> p h d", d=DH),
                        )
                    units.append(uv)
                return units

            ycnt = [0]

            def outproj_units(m):
                """PE work units projecting OT rows of chunk m into y.
                Both 512-col halves accumulate into the two pss slots, then one
                fused DVE copy (bank-strided 3D AP) and one DMA emit the row."""
                units = []
                for itl in range(TPC):
                    def u(itl=itl):
                        it = TPC * m + itl
                        psy = pss.tile([P, 2, MC], F32, tag=f"s{rot[0] % 2}",
                                       name=f"y{it}")
                        rot[0] += 1
                        for nc2 in range(D // MC):
                            for pr2 in range(NPR):
                                nc.tensor.matmul(
                                    psy[:, nc2, :],
                                    lhsT=OT[m][:, pr2, itl * P : itl * P + P],
                                    rhs=wo_sb[:, pr2, nc2 * MC : nc2 * MC + MC],
                                    start=(pr2 == 0),
                                    stop=(pr2 == NPR - 1),
                                    skip_group_check=True,
                                )
                        y_t = tmp.tile([P, D], BF16, tag="ysb", name=f"ysb{it}")
                        nc.vector.tensor_copy(
                            y_t[:].rearrange("p (two c) -> p two c", two=2),
                            psy[:],
                        )
                        nc.sync.dma_start(y[it * P : it * P + P, :], y_t[:])
                    units.append(u)
                return units

            def emit_pv(po, m, pr, t, ptb, rs, is_last):
                p0 = MC * m
                w = p0 + MC - rs
                mck, tl = t // TPC, t % TPC
                for h2 in range(2):
                    nc.tensor.matmul(
                        po[0 : DH + 1, h2, rs - p0 : MC],
                        lhsT=Vp[mck][:, tl, 2 * pr + h2, 0 : DH + 1],
                        rhs=ptb[:, h2, 0:w],
                        start=(t == 0),
                        stop=is_last,
                        skip_group_check=True,
                    )

            def normalize(po, m, pr):
                # OT = po[0:64] * (1 / po[64]); both heads fused via the
                # [*, 2, MC] po tile spanning two adjacent banks.
                l_sb = small.tile([1, 2, MC], F32, tag="lsb")
                nc.vector.tensor_copy(l_sb[:], po[DH : DH + 1, :, :])
                r32 = small.tile([1, 2, MC], F32, tag="r32")
                nc.vector.reciprocal_approx_fast(r32[:], l_sb[:])
                r16 = small.tile([1, 2, MC], BF16, tag="r16")
                nc.gpsimd.tensor_copy(r16[:], r32[:])
                if debug:
                    nc.sync.dma_start(dr32[m, pr, :, :], r32[0, :, :])
                rb = small.tile([DH, 2, MC], BF16, tag="rb")
                nc.gpsimd.partition_broadcast(rb[:], r16[:])
                for h2 in range(2):
                    nc.vector.tensor_mul(
                        OT[m][DH * h2 : DH * h2 + DH, pr, :],
                        po[0:DH, h2, :], rb[:, h2, :],
                    )

            carry = []  # deferred last-PV + normalize from the previous segment

            def attention_pass(m, filler):
                p0 = MC * m
                ntile = TPC * (m + 1)
                total_rounds = 2 * ntile
                nf = len(filler)
                fi = 0
                rd = 0
                for pr in range(NPR):
                    par = (2 * m + pr) % 2
                    po = pop.tile([P, 2, MC], F32, tag=f"o{par}",
                                  name=f"o{m}_{pr}")
                    pend = None
                    for t in range(ntile):
                        mck, tl = t // TPC, t % TPC
                        rs = max(P * t, p0)
                        off = rs - p0
                        buf = pss.tile([P, 2, MC], F32, tag=f"s{t % 2}",
                                       name=f"sc{m}_{pr}_{t}")
                        for h2 in range(2):
                            nc.tensor.matmul(
                                buf[:, h2, off:MC],
                                lhsT=KTt[pr][mck][DH * h2 : DH * h2 + DH,
                                                  tl * P : tl * P + P],
                                rhs=QT[pr][m][DH * h2 : DH * h2 + DH, off:MC],
                                start=True, stop=True,
                                skip_group_check=True,
                            )
                        ptb_cur = pts.tile([P, 2, MC], BF16, tag="pt",
                                           name=f"pt{m}_{pr}_{t}")
                        nc.scalar.activation(
                            ptb_cur[:, :, 0 : MC - off],
                            buf[:, :, off:MC],
                            AF.Exp, scale=0.125, bias=bias_sb[:],
                        )
                        if rs == P * t:  # diagonal tile: causal 0/1 mask
                            nc.gpsimd.tensor_mul(
                                ptb_cur[:, :, 0:P],
                                ptb_cur[:, :, 0:P],
                                trib_sb[:],
                            )
                        if pend is not None:
                            emit_pv(*pend)
                            pend = None
                        elif carry:
                            carry.pop(0)()
                        pend = (po, m, pr, t, ptb_cur, rs, t == ntile - 1)
                        rd += 1
                        while fi < nf and fi * total_rounds < rd * nf:
                            filler[fi]()
                            fi += 1
                    def boundary(po=po, m=m, pr=pr, pend=pend):
                        emit_pv(*pend)
                        normalize(po, m, pr)
                    carry.append(boundary)
                while fi < nf:
                    filler[fi]()
                    fi += 1

            # ---------------- main schedule ----------------
            # prologue: interleave weight and chunk-0 DMAs in consumption order
            xq_t = xin.tile([P, KT, MC], BF16, tag="xq", name="xq0")
            xk_t = xin.tile([P, KT, MC], BF16, tag="xk", name="xk0")
            xv_t = xin.tile([P, KT, MC], BF16, tag="xv", name="xv0")
            nc.sync.dma_start(wq_sb[:], wq[:].rearrange("(kt p) n -> p kt n", p=P))
            nc.sync.dma_start(xq_t[:], xq[0].rearrange("(kt p) c -> p kt c", p=P))
            nc.sync.dma_start(trib_sb[:], trib[:])
            nc.sync.dma_start(c_sb[0][:], ctab[0].rearrange("pr p c -> p pr c"))
            nc.sync.dma_start(s_sb[0][:], stab[0].rearrange("pr p c -> p pr c"))
            nc.sync.dma_start(wk_sb[:], wk[:].rearrange("(kt p) n -> p kt n", p=P))
            nc.sync.dma_start(xk_t[:], xk[0].rearrange("(kt p) c -> p kt c", p=P))
            nc.sync.dma_start(wv_sb[:], wv[:].rearrange("(kt p) n -> p kt n", p=P))
            nc.sync.dma_start(xv_t[:], xv[0].rearrange("(kt p) c -> p kt c", p=P))
            nc.sync.dma_start(wo_sb[:], wo[:].rearrange("(pr p) n -> p pr n", p=P))
            if qk_bias:
                nc.sync.dma_start(rq_sb[0][:], rqt[0].rearrange("pr p c -> p pr c"))
                nc.sync.dma_start(rk_sb[0][:], rkt[0].rearrange("pr p c -> p pr c"))

            for u in proj_units(0, xq_t, xk_t, xv_t):
                u()

            for m in range(NCH):
                filler = []
                if m + 1 < NCH:
                    x_t = dma_chunk(m + 1)
                    filler += proj_units(m + 1, *x_t)
                if m >= 1:
                    filler += outproj_units(m - 1)
                attention_pass(m, filler)
                if debug:
                    nc.sync.dma_start(dot[m], OT[m][:])
            while carry:
                carry.pop(0)()
            for u in outproj_units(NCH - 1):
                u()
    nc.compile()
    return nc


@functools.lru_cache(maxsize=2)
def _get_program(L, qk_bias=False):
    return build_program(L, qk_bias)


def _rope_perm(hloc):
    """Column order (within this core's 256 outputs) for head-local index hloc.

    Row r (0..63) of head h: quadrant q = r//32, i = r%32.
    i < 16  -> even dim of freq 16q+i       (a half)
    i >= 16 -> odd dim  of freq 16q+(i-16)  (b half)
    Returns indices into the head's 64 original dims.
    """
    idx = np.zeros(64, dtype=np.int64)
    for r in range(64):
        q, i = divmod(r, 32)
        if i < 16:
            idx[r] = 2 * (16 * q + i)
        else:
            idx[r] = 2 * (16 * q + (i - 16)) + 1
    return idx


def _prep_core_inputs(c, L, q, k, v, Wq, bq, Wk, bk, Wv, bv, Wo, bo):
    b = c // (N_CORES // B)
    hb = HPC * (c % (N_CORES // B))  # first global head on this core
    bf = ml_dtypes.bfloat16
    f8 = mybir.dt.np(FP8)
    NCH = L // MC

    def chunked(x2d, dt):  # [D, L] -> [NCH, D, MC]
        return np.ascontiguousarray(
            x2d.reshape(D, NCH, MC).transpose(1, 0, 2)
        ).astype(dt)

    xq = chunked(q[b].T, bf)
    xk = chunked(k[b].T, bf)
    xv8 = chunked(v[b].T, bf)

    # permuted row order of Wq/Wk for this core's 4 heads
    rows = np.concatenate(
        [64 * (hb + hl) + _rope_perm(hl) for hl in range(HPC)]
    )
    wq_t = np.ascontiguousarray(Wq[rows, :].T).astype(bf)        # [D, 256]
    wk_t = np.ascontiguousarray(Wk[rows, :].T).astype(bf)
    bq_p = bq[rows].astype(np.float64).reshape(NPR, P)
    bk_p = bk[rows].astype(np.float64).reshape(NPR, P)
    vrows = np.arange(64 * hb, 64 * (hb + HPC))
    wv_t = np.ascontiguousarray(Wv[vrows, :].T).astype(bf)       # [D, 256]
    wo_t = np.ascontiguousarray(Wo[:, vrows].T).astype(bf)       # [256, D]

    # rope tables in permuted row order; sin negated on b halves
    pos = np.arange(L, dtype=np.float64)
    ct = np.zeros((NPR, P, L), dtype=np.float64)
    st = np.zeros((NPR, P, L), dtype=np.float64)
    for pr in range(NPR):
        for h2 in range(2):
            hg = hb + 2 * pr + h2
            for r in range(64):
                qd, i = divmod(r, 32)
                f = 32 * hg + 16 * qd + (i % 16)
                theta = MAXPOS ** (-f / (D // 2))
                ang = pos * theta
                row = DH * h2 + r
                ct[pr, row] = np.cos(ang)
                st[pr, row] = np.sin(ang) if i < 16 else -np.sin(ang)

    def tchunk(a):  # [NPR, P, L] -> [NCH, NPR, P, MC]
        return np.ascontiguousarray(
            a.reshape(NPR, P, NCH, MC).transpose(2, 0, 1, 3)
        ).astype(np.float32).astype(bf)

    ctab = tchunk(ct)
    stab = tchunk(st)

    jj = np.arange(P)
    tri1 = np.where(jj[None, :] >= jj[:, None], 1.0, 0.0)  # [k, q] keep q>=k
    trib = np.ascontiguousarray(
        np.broadcast_to(tri1[:, None, :], (P, 2, P))
    ).astype(np.float32).astype(bf)

    im = {
        "xq": xq, "xk": xk, "xv": xv8,
        "wq": wq_t, "wk": wk_t, "wv": wv_t, "wo": wo_t,
        "ctab": ctab, "stab": stab, "trib": trib,
    }
    if np.abs(bq).max() > 0 or np.abs(bk).max() > 0:
        def swap16(a):
            a4 = a.reshape(NPR, P // 32, 2, 16, L)
            return a4[:, :, ::-1, :, :].reshape(NPR, P, L)

        rqt = bq_p[:, :, None] * ct + swap16(bq_p[:, :, None] * st)
        rkt = bk_p[:, :, None] * ct + swap16(bk_p[:, :, None] * st)
        im["rqt"] = tchunk(rqt)
        im["rkt"] = tchunk(rkt)
    return im


def kernel(q, k, v, Wq, bq, Wk, bk, Wv, bv, Wo, bo):
    q, k, v = (np.asarray(a, dtype=np.float32) for a in (q, k, v))
    Wq, bq, Wk, bk, Wv, bv, Wo, bo = (
        np.asarray(a, dtype=np.float32) for a in (Wq, bq, Wk, bk, Wv, bv, Wo, bo)
    )
    Bq, L, Dq = q.shape
    assert (Bq, Dq) == (B, D)

    qk_bias = bool(np.abs(bq).max() > 0 or np.abs(bk).max() > 0)
    nc = _get_program(L, qk_bias)
    in_maps = [
        _prep_core_inputs(c, L, q, k, v, Wq, bq, Wk, bk, Wv, bv, Wo, bo)
        for c in range(N_CORES)
    ]
    res = run_bass_kernel_spmd(nc, in_maps, core_ids=list(range(N_CORES)))

    corr = (bv @ Wo.T + bo).astype(np.float32)  # folded-out V/O biases
    y = np.zeros((B, L, D), dtype=np.float32)
    cpb = N_CORES // B
    for c in range(N_CORES):
        y[c // cpb] += np.asarray(res.results[c]["y"], dtype=np.float32)
    y += corr[None, None, :]
    return y


# revision 16
# speedup vs baseline: 1.3630x; 1.3630x over previous
"""Trainium2 Bass kernel for causal multi-head attention with full-dim rotary.

Computes, for inputs q,k,v [B=2, L=2048, D=1024] and weights Wq/Wk/Wv/Wo [D,D]:
    Q = rope(q @ Wq.T + bq); K = rope(k @ Wk.T + bk); V = v @ Wv.T + bv
    out = softmax_causal(Qh Kh^T / sqrt(dh)) Vh  (H=16 heads, dh=64)
    y = out @ Wo.T + bo

Sharding: 8 cores = (batch b in {0,1}) x (block of 4 heads). Each core computes
its 4 heads' Q/K/V projections (256 output features), runs causal flash
attention in S^T orientation, and emits a partial output projection y [L, D].
The host sums the 4 partials per batch and adds the bias correction row.

v2 architecture (software-pipelined super-steps over 512-row chunks):
  pass m attention (ACT-bound exp stream) overlaps the PE-side projection of
  chunk m+1 and out-projection of rows from pass m-1, woven into the round
  loop so the tensor engine never idles long enough to re-throttle (HAM).
  Both heads of a pair share one exp instruction ([128, 2, w] across two PSUM
  banks). All matmuls are bf16 (fp8 fails the max-error budget here). The
  softmax denominator is fused as a ones-column of V'; the causal mask is a
  -1000 pre-exp bias added onto the diagonal score blocks; exp carries a -3
  shift (cancels in the softmax ratio).
"""

import sys
import functools
import numpy as np

try:
    import concourse.bass as bass
except ImportError:  # fresh grading dir: concourse lives in the container image
    sys.path.insert(0, "/opt/trn_rl_repo")
    import concourse.bass as bass

import ml_dtypes
import concourse.mybir as mybir
import concourse.tile as tile
from concourse import bacc
from concourse.bass_utils import run_bass_kernel_spmd

BF16 = mybir.dt.bfloat16
F32 = mybir.dt.float32
FP8 = mybir.dt.float8e4
AF = mybir.ActivationFunctionType
DR = mybir.MatmulPerfMode.DoubleRow

B, D, H, DH = 2, 1024, 16, 64
P = 128
KT = D // P            # 8 contraction tiles for the projections
HPC = 4                # heads per core
NPR = 2                # head-pairs per core
N_CORES = 8
MAXPOS = 10000.0
MC = 512               # chunk / pass width
TPC = MC // P          # key tiles per chunk
VW = 72                # padded per-head V' width (64 dims + ones + pad)
EXP_SHIFT = -3.0
MASK_BIAS = -1000.0


def build_program(L, qk_bias=False, debug=False):
    NCH = L // MC
    NT = L // P

    nc = bacc.Bacc(None)
    xq = nc.declare_dram_parameter("xq", [NCH, D, MC], BF16, isOutput=False)
    xk = nc.declare_dram_parameter("xk", [NCH, D, MC], BF16, isOutput=False)
    xv = nc.declare_dram_parameter("xv", [NCH, D, MC], BF16, isOutput=False)
    wq = nc.declare_dram_parameter("wq", [D, 2 * P], BF16, isOutput=False)
    wk = nc.declare_dram_parameter("wk", [D, 2 * P], BF16, isOutput=False)
    wv = nc.declare_dram_parameter("wv", [D, 2 * P], BF16, isOutput=False)
    wo = nc.declare_dram_parameter("wo", [2 * P, D], BF16, isOutput=False)
    ctab = nc.declare_dram_parameter("ctab", [NCH, NPR, P, MC], BF16, isOutput=False)
    stab = nc.declare_dram_parameter("stab", [NCH, NPR, P, MC], BF16, isOutput=False)
    if qk_bias:
        rqt = nc.declare_dram_parameter("rqt", [NCH, NPR, P, MC], BF16, isOutput=False)
        rkt = nc.declare_dram_parameter("rkt", [NCH, NPR, P, MC], BF16, isOutput=False)
    trib = nc.declare_dram_parameter("trib", [P, 2, P], BF16, isOutput=False)
    y = nc.declare_dram_parameter("y", [L, D], BF16, isOutput=True)
    if debug:
        dot = nc.declare_dram_parameter("dot", [NCH, P, NPR, MC], BF16,
                                        isOutput=True)
        dr32 = nc.declare_dram_parameter("dr32", [NCH, NPR, 2, MC], F32,
                                         isOutput=True)

    with tile.TileContext(nc) as tc:
        from contextlib import ExitStack

        with ExitStack() as ctx:
            consts = ctx.enter_context(tc.tile_pool(name="consts", bufs=1))
            qk_sb = ctx.enter_context(tc.tile_pool(name="qk_sb", bufs=1))
            xin = ctx.enter_context(tc.tile_pool(name="xin", bufs=2))
            tmp = ctx.enter_context(tc.tile_pool(name="tmp", bufs=3))
            pts = ctx.enter_context(tc.tile_pool(name="pts", bufs=3))
            small = ctx.enter_context(tc.tile_pool(name="small", bufs=2))
            pss = ctx.enter_context(tc.tile_pool(name="pss", bufs=1, space="PSUM"))
            pop = ctx.enter_context(tc.tile_pool(name="pop", bufs=1, space="PSUM"))

            # ---- constant/persistent tiles ----
            wq_sb = consts.tile([P, KT, 2 * P], BF16, tag="wq")
            wk_sb = consts.tile([P, KT, 2 * P], BF16, tag="wk")
            wv_sb = consts.tile([P, KT, 2 * P], BF16, tag="wv")
            wo_sb = consts.tile([P, NPR, D], BF16, tag="wo")
            trib_sb = consts.tile([P, 2, P], BF16, tag="trib")
            bias_sb = consts.tile([P, 1], F32, tag="bias")
            nc.vector.memset(bias_sb[:], EXP_SHIFT)

            c_sb = [consts.tile([P, NPR, MC], BF16, tag=f"c{m}", name=f"c{m}")
                    for m in range(NCH)]
            s_sb = [consts.tile([P, NPR, MC], BF16, tag=f"s{m}", name=f"s{m}")
                    for m in range(NCH)]
            if qk_bias:
                rq_sb = [consts.tile([P, NPR, MC], BF16, tag=f"rq{m}", name=f"rq{m}")
                         for m in range(NCH)]
                rk_sb = [consts.tile([P, NPR, MC], BF16, tag=f"rk{m}", name=f"rk{m}")
                         for m in range(NCH)]

            QT = [[qk_sb.tile([P, MC], BF16, tag=f"QT{pr}_{m}", name=f"QT{pr}_{m}")
                   for m in range(NCH)] for pr in range(NPR)]
            KTt = [[qk_sb.tile([P, MC], BF16, tag=f"KT{pr}_{m}", name=f"KT{pr}_{m}")
                    for m in range(NCH)] for pr in range(NPR)]
            OT = [qk_sb.tile([P, NPR, MC], BF16, tag=f"OT_{m}", name=f"OT_{m}")
                  for m in range(NCH)]
            # V' per chunk: [keys, tile, head, 64 dims + ones]
            Vp = [qk_sb.tile([P, TPC, HPC, DH + 1], BF16, tag=f"Vp{m}",
                             name=f"Vp{m}") for m in range(NCH)]
            for m in range(NCH):
                nc.vector.memset(Vp[m][:, :, :, DH : DH + 1], 1.0)

            # a<->b half swap within each 32-partition quadrant
            SWAP = [(i + 16) % 32 for i in range(32)]

            # ---------------- emission helpers ----------------
            def dma_chunk(m):
                xq_t = xin.tile([P, KT, MC], BF16, tag="xq", name=f"xq{m}")
                xk_t = xin.tile([P, KT, MC], BF16, tag="xk", name=f"xk{m}")
                xv_t = xin.tile([P, KT, MC], BF16, tag="xv", name=f"xv{m}")
                nc.sync.dma_start(xq_t[:], xq[m].rearrange("(kt p) c -> p kt c", p=P))
                nc.sync.dma_start(c_sb[m][:], ctab[m].rearrange("pr p c -> p pr c"))
                nc.sync.dma_start(s_sb[m][:], stab[m].rearrange("pr p c -> p pr c"))
                nc.sync.dma_start(xk_t[:], xk[m].rearrange("(kt p) c -> p kt c", p=P))
                nc.sync.dma_start(xv_t[:], xv[m].rearrange("(kt p) c -> p kt c", p=P))
                if qk_bias:
                    nc.sync.dma_start(rq_sb[m][:],
                                      rqt[m].rearrange("pr p c -> p pr c"))
                    nc.sync.dma_start(rk_sb[m][:],
                                      rkt[m].rearrange("pr p c -> p pr c"))
                return xq_t, xk_t, xv_t

            rot = [0]  # pss-slot borrow rotation (proj + outproj psy)

            def proj_units(m, xq_t, xk_t, xv_t):
                """PE work units projecting chunk m into QT/KTt/Vp.
                rope: sb = bf16(ps); t2s = shuffle(sb*sin); QT = sb*cos + t2s
                (multiplies and add on gpsimd, shuffle on DVE)."""
                units = []
                for name, w_sb, x_t, dst in (
                    ("q", wq_sb, xq_t, QT),
                    ("k", wk_sb, xk_t, KTt),
                ):
                    for pr in range(NPR):
                        def u(pr=pr, name=name, w_sb=w_sb, x_t=x_t, dst=dst):
                            ps = pss.tile([P, MC], F32, tag=f"s{rot[0] % 2}",
                                          name=f"pj{name}{m}_{pr}")
                            rot[0] += 1
                            for kt in range(KT):
                                nc.tensor.matmul(
                                    ps[:],
                                    lhsT=w_sb[:, kt, pr * P : pr * P + P],
                                    rhs=x_t[:, kt, :],
                                    start=(kt == 0),
                                    stop=(kt == KT - 1),
                                    skip_group_check=True,
                                )
                            t2 = tmp.tile([P, MC], BF16, tag="t2")
                            t2s = tmp.tile([P, MC], BF16, tag="t2s")
                            t1 = tmp.tile([P, MC], BF16, tag="t1")
                            nc.vector.tensor_mul(t2[:], ps[:], s_sb[m][:, pr, :])
                            nc.vector.tensor_mul(t1[:], ps[:], c_sb[m][:, pr, :])
                            nc.vector.stream_shuffle(t2s[:], t2[:], SWAP)
                            if qk_bias:
                                r_sb = rq_sb if name == "q" else rk_sb
                                t3 = tmp.tile([P, MC], BF16, tag="t3")
                                nc.gpsimd.tensor_add(t3[:], t1[:], t2s[:])
                                nc.gpsimd.tensor_add(
                                    dst[pr][m][:], t3[:], r_sb[m][:, pr, :]
                                )
                            else:
                                nc.gpsimd.tensor_add(dst[pr][m][:], t1[:], t2s[:])
                        units.append(u)
                for u2 in range(TPC // 2):
                    def uv(u2=u2):
                        ps = pss.tile([P, MC], F32, tag=f"s{rot[0] % 2}",
                                      name=f"pjv{m}_{u2}")
                        rot[0] += 1
                        for half in range(2):
                            psv = ps[:, 2 * P * half : 2 * P * half + 2 * P]
                            msub = 2 * u2 + half
                            for kt in range(KT):
                                nc.tensor.matmul(
                                    psv,
                                    lhsT=xv_t[:, kt, msub * P : msub * P + P],
                                    rhs=wv_sb[:, kt, :],
                                    start=(kt == 0),
                                    stop=(kt == KT - 1),
                                    skip_group_check=True,
                                )
                        nc.vector.tensor_copy(
                            Vp[m][:, 2 * u2 : 2 * u2 + 2, :, 0:DH],
                            ps[:].rearrange("p (two h d) -> p two h d",
                                            two=2, d=DH),
                        )
                    units.append(uv)
                return units

            ycnt = [0]

            def outproj_units(m):
                """PE work units projecting OT rows of chunk m into y.
                Both 512-col halves accumulate into the two pss slots, then one
                fused DVE copy (bank-strided 3D AP) and one DMA emit the row."""
                units = []
                for itl in range(TPC):
                    def u(itl=itl):
                        it = TPC * m + itl
                        psy = pss.tile([P, 2, MC], F32, tag=f"s{rot[0] % 2}",
                                       name=f"y{it}")
                        rot[0] += 1
                        for nc2 in range(D // MC):
                            for pr2 in range(NPR):
                                nc.tensor.matmul(
                                    psy[:, nc2, :],
                                    lhsT=OT[m][:, pr2, itl * P : itl * P + P],
                                    rhs=wo_sb[:, pr2, nc2 * MC : nc2 * MC + MC],
                                    start=(pr2 == 0),
                                    stop=(pr2 == NPR - 1),
                                    skip_group_check=True,
                                )
                        y_t = tmp.tile([P, D], BF16, tag="ysb", name=f"ysb{it}")
                        nc.vector.tensor_copy(
                            y_t[:].rearrange("p (two c) -> p two c", two=2),
                            psy[:],
                        )
                        nc.sync.dma_start(y[it * P : it * P + P, :], y_t[:])
                    units.append(u)
                return units

            def emit_pv(po, m, pr, t, ptb, rs, is_last):
                p0 = MC * m
                w = p0 + MC - rs
                mck, tl = t // TPC, t % TPC
                for h2 in range(2):
                    nc.tensor.matmul(
                        po[0 : DH + 1, h2, rs - p0 : MC],
                        lhsT=Vp[mck][:, tl, 2 * pr + h2, 0 : DH + 1],
                        rhs=ptb[:, h2, 0:w],
                        start=(t == 0),
                        stop=is_last,
                        skip_group_check=True,
                    )

            def normalize(po, m, pr):
                # OT = po[0:64] * (1 / po[64]); both heads fused via the
                # [*, 2, MC] po tile spanning two adjacent banks.
                l_sb = small.tile([1, 2, MC], F32, tag="lsb")
                nc.vector.tensor_copy(l_sb[:], po[DH : DH + 1, :, :])
                r32 = small.tile([1, 2, MC], F32, tag="r32")
                nc.vector.reciprocal_approx_fast(r32[:], l_sb[:])
                r16 = small.tile([1, 2, MC], BF16, tag="r16")
                nc.vector.tensor_copy(r16[:], r32[:])
                if debug:
                    nc.sync.dma_start(dr32[m, pr, :, :], r32[0, :, :])
                rb = small.tile([DH, 2, MC], BF16, tag="rb")
                nc.gpsimd.partition_broadcast(rb[:], r16[:])
                for h2 in range(2):
                    nc.vector.tensor_mul(
                        OT[m][DH * h2 : DH * h2 + DH, pr, :],
                        po[0:DH, h2, :], rb[:, h2, :],
                    )

            carry = []  # deferred last-PV + normalize from the previous segment

            def attention_pass(m, filler):
                p0 = MC * m
                ntile = TPC * (m + 1)
                total_rounds = 2 * ntile
                nf = len(filler)
                fi = 0
                rd = 0
                for pr in range(NPR):
                    par = (2 * m + pr) % 2
                    po = pop.tile([P, 2, MC], F32, tag=f"o{par}",
                                  name=f"o{m}_{pr}")
                    pend = None
                    for t in range(ntile):
                        mck, tl = t // TPC, t % TPC
                        rs = max(P * t, p0)
                        off = rs - p0
                        buf = pss.tile([P, 2, MC], F32, tag=f"s{t % 2}",
                                       name=f"sc{m}_{pr}_{t}")
                        for h2 in range(2):
                            nc.tensor.matmul(
                                buf[:, h2, off:MC],
                                lhsT=KTt[pr][mck][DH * h2 : DH * h2 + DH,
                                                  tl * P : tl * P + P],
                                rhs=QT[pr][m][DH * h2 : DH * h2 + DH, off:MC],
                                start=True, stop=True,
                                skip_group_check=True,
                            )
                        ptb_cur = pts.tile([P, 2, MC], BF16, tag="pt",
                                           name=f"pt{m}_{pr}_{t}")
                        nc.scalar.activation(
                            ptb_cur[:, :, 0 : MC - off],
                            buf[:, :, off:MC],
                            AF.Exp, scale=0.125, bias=bias_sb[:],
                        )
                        if rs == P * t:  # diagonal tile: causal 0/1 mask
                            nc.gpsimd.tensor_mul(
                                ptb_cur[:, :, 0:P],
                                ptb_cur[:, :, 0:P],
                                trib_sb[:],
                            )
                        if pend is not None:
                            emit_pv(*pend)
                            pend = None
                        elif carry:
                            carry.pop(0)()
                        pend = (po, m, pr, t, ptb_cur, rs, t == ntile - 1)
                        rd += 1
                        while fi < nf and fi * total_rounds < rd * nf:
                            filler[fi]()
                            fi += 1
                    def boundary(po=po, m=m, pr=pr, pend=pend):
                        emit_pv(*pend)
                        normalize(po, m, pr)
                    carry.append(boundary)
                while fi < nf:
                    filler[fi]()
                    fi += 1

            # ---------------- main schedule ----------------
            # prologue: interleave weight and chunk-0 DMAs in consumption order
            xq_t = xin.tile([P, KT, MC], BF16, tag="xq", name="xq0")
            xk_t = xin.tile([P, KT, MC], BF16, tag="xk", name="xk0")
            xv_t = xin.tile([P, KT, MC], BF16, tag="xv", name="xv0")
            nc.sync.dma_start(wq_sb[:], wq[:].rearrange("(kt p) n -> p kt n", p=P))
            nc.sync.dma_start(xq_t[:], xq[0].rearrange("(kt p) c -> p kt c", p=P))
            nc.sync.dma_start(trib_sb[:], trib[:])
            nc.sync.dma_start(c_sb[0][:], ctab[0].rearrange("pr p c -> p pr c"))
            nc.sync.dma_start(s_sb[0][:], stab[0].rearrange("pr p c -> p pr c"))
            nc.sync.dma_start(wk_sb[:], wk[:].rearrange("(kt p) n -> p kt n", p=P))
            nc.sync.dma_start(xk_t[:], xk[0].rearrange("(kt p) c -> p kt c", p=P))
            nc.sync.dma_start(wv_sb[:], wv[:].rearrange("(kt p) n -> p kt n", p=P))
            nc.sync.dma_start(xv_t[:], xv[0].rearrange("(kt p) c -> p kt c", p=P))
            nc.sync.dma_start(wo_sb[:], wo[:].rearrange("(pr p) n -> p pr n", p=P))
            if qk_bias:
                nc.sync.dma_start(rq_sb[0][:], rqt[0].rearrange("pr p c -> p pr c"))
                nc.sync.dma_start(rk_sb[0][:], rkt[0].rearrange("pr p c -> p pr c"))

            for u in proj_units(0, xq_t, xk_t, xv_t):
                u()

            for m in range(NCH):
                filler = []
                if m + 1 < NCH:
                    x_t = dma_chunk(m + 1)
                    filler += proj_units(m + 1, *x_t)
                if m >= 1:
                    filler += outproj_units(m - 1)
                attention_pass(m, filler)
                if debug:
                    nc.sync.dma_start(dot[m], OT[m][:])
            while carry:
                carry.pop(0)()
            for u in outproj_units(NCH - 1):
                u()
    nc.compile()
    return nc


@functools.lru_cache(maxsize=2)
def _get_program(L, qk_bias=False):
    return build_program(L, qk_bias)


def _rope_perm(hloc):
    """Column order (within this core's 256 outputs) for head-local index hloc.

    Row r (0..63) of head h: quadrant q = r//32, i = r%32.
    i < 16  -> even dim of freq 16q+i       (a half)
    i >= 16 -> odd dim  of freq 16q+(i-16)  (b half)
    Returns indices into the head's 64 original dims.
    """
    idx = np.zeros(64, dtype=np.int64)
    for r in range(64):
        q, i = divmod(r, 32)
        if i < 16:
            idx[r] = 2 * (16 * q + i)
        else:
            idx[r] = 2 * (16 * q + (i - 16)) + 1
    return idx


def _prep_core_inputs(c, L, q, k, v, Wq, bq, Wk, bk, Wv, bv, Wo, bo):
    b = c // (N_CORES // B)
    hb = HPC * (c % (N_CORES // B))  # first global head on this core
    bf = ml_dtypes.bfloat16
    f8 = mybir.dt.np(FP8)
    NCH = L // MC

    def chunked(x2d, dt):  # [D, L] -> [NCH, D, MC]
        return np.ascontiguousarray(
            x2d.reshape(D, NCH, MC).transpose(1, 0, 2)
        ).astype(dt)

    xq = chunked(q[b].T, bf)
    xk = chunked(k[b].T, bf)
    xv8 = chunked(v[b].T, bf)

    # permuted row order of Wq/Wk for this core's 4 heads
    rows = np.concatenate(
        [64 * (hb + hl) + _rope_perm(hl) for hl in range(HPC)]
    )
    wq_t = np.ascontiguousarray(Wq[rows, :].T).astype(bf)        # [D, 256]
    wk_t = np.ascontiguousarray(Wk[rows, :].T).astype(bf)
    bq_p = bq[rows].astype(np.float64).reshape(NPR, P)
    bk_p = bk[rows].astype(np.float64).reshape(NPR, P)
    vrows = np.arange(64 * hb, 64 * (hb + HPC))
    wv_t = np.ascontiguousarray(Wv[vrows, :].T).astype(bf)       # [D, 256]
    wo_t = np.ascontiguousarray(Wo[:, vrows].T).astype(bf)       # [256, D]

    # rope tables in permuted row order; sin negated on b halves
    pos = np.arange(L, dtype=np.float64)
    ct = np.zeros((NPR, P, L), dtype=np.float64)
    st = np.zeros((NPR, P, L), dtype=np.float64)
    for pr in range(NPR):
        for h2 in range(2):
            hg = hb + 2 * pr + h2
            for r in range(64):
                qd, i = divmod(r, 32)
                f = 32 * hg + 16 * qd + (i % 16)
                theta = MAXPOS ** (-f / (D // 2))
                ang = pos * theta
                row = DH * h2 + r
                ct[pr, row] = np.cos(ang)
                st[pr, row] = np.sin(ang) if i < 16 else -np.sin(ang)

    def tchunk(a):  # [NPR, P, L] -> [NCH, NPR, P, MC]
        return np.ascontiguousarray(
            a.reshape(NPR, P, NCH, MC).transpose(2, 0, 1, 3)
        ).astype(np.float32).astype(bf)

    ctab = tchunk(ct)
    stab = tchunk(st)

    jj = np.arange(P)
    tri1 = np.where(jj[None, :] >= jj[:, None], 1.0, 0.0)  # [k, q] keep q>=k
    trib = np.ascontiguousarray(
        np.broadcast_to(tri1[:, None, :], (P, 2, P))
    ).astype(np.float32).astype(bf)

    im = {
        "xq": xq, "xk": xk, "xv": xv8,
        "wq": wq_t, "wk": wk_t, "wv": wv_t, "wo": wo_t,
        "ctab": ctab, "stab": stab, "trib": trib,
    }
    if np.abs(bq).max() > 0 or np.abs(bk).max() > 0:
        def swap16(a):
            a4 = a.reshape(NPR, P // 32, 2, 16, L)
            return a4[:, :, ::-1, :, :].reshape(NPR, P, L)

        rqt = bq_p[:, :, None] * ct + swap16(bq_p[:, :, None] * st)
        rkt = bk_p[:, :, None] * ct + swap16(bk_p[:, :, None] * st)
        im["rqt"] = tchunk(rqt)
        im["rkt"] = tchunk(rkt)
    return im


def kernel(q, k, v, Wq, bq, Wk, bk, Wv, bv, Wo, bo):
    q, k, v = (np.asarray(a, dtype=np.float32) for a in (q, k, v))
    Wq, bq, Wk, bk, Wv, bv, Wo, bo = (
        np.asarray(a, dtype=np.float32) for a in (Wq, bq, Wk, bk, Wv, bv, Wo, bo)
    )
    Bq, L, Dq = q.shape
    assert (Bq, Dq) == (B, D)

    qk_bias = bool(np.abs(bq).max() > 0 or np.abs(bk).max() > 0)
    nc = _get_program(L, qk_bias)
    in_maps = [
        _prep_core_inputs(c, L, q, k, v, Wq, bq, Wk, bk, Wv, bv, Wo, bo)
        for c in range(N_CORES)
    ]
    res = run_bass_kernel_spmd(nc, in_maps, core_ids=list(range(N_CORES)))

    corr = (bv @ Wo.T + bo).astype(np.float32)  # folded-out V/O biases
    y = np.zeros((B, L, D), dtype=np.float32)
    cpb = N_CORES // B
    for c in range(N_CORES):
        y[c // cpb] += np.asarray(res.results[c]["y"], dtype=np.float32)
    y += corr[None, None, :]
    return y
